# revision 43
# baseline (speedup 1.0000x reference)
"""DigitCaps dynamic-routing kernel for 8 TRN2 NeuronCores (v2).

Problem (hardcoded): x [256,1152,8] f32, W [1,1152,10,16,8] f32, 3 routing
iterations -> v [256,10,16,1] f32.

Strategy: shard the R=1152 routes 8-ways (144 per core), full batch B=256 on
every core. u_hat is never materialized; each iteration streams W through the
TensorEngine:
  s_c[o,b]   = sum_{(r,i)} Ws_c[(r,i),(c,o)] * (en_c[r,b] * x[(r,i),b])  (PE)
  (AllReduce s over the 8 R-shards in [CO,B] layout, squash -> v)
  M_c[b,(r,i)] = sum_o v_c[o,b] * WoT_c[o,(r,i)]                         (PE)
  a_c[b,r]   = sum_i x[b,(r,i)] * M_c[b,(r,i)]                           (DVE)

v2 changes vs baseline:
- AllReduce bounce kept in [CO,B] layout (contiguous descriptors; the old
  transposed write emitted ~41k 4-byte descriptors / 100us per phase).
- v lives in [co,b]; v^T for the M-matmul is 10 small realign DMAs; the
  whole v_transpose PE phase is gone. Output transposed once at the end.
- softmax without max-shift (logits are bounded ~+-30, exp is f32-safe);
  z-reciprocal applied via step-0 broadcast AP (no zrep materialization).
- en in bf16 (measured end-to-end impact ~5e-3); en-transpose via normal
  matmul against a bf16 identity (1cyc/row vs 4 for fp32 LOW_HIGH).
- M-path (a-phase) in bf16, s-path f32 except the last iteration (bf16);
  measured combined rel err ~6e-3 vs the 2e-2 gate.
- dummy 4-byte AllReduce issued first to absorb the cc entry barrier /
  ncfw warmup under the input load.
- engine spread: psum drains and big elementwise ops split across
  Scalar/Vector/GpSimd so no single engine serializes; GpSimd kept free
  near collective triggers.
"""

import sys

if "/opt/trn_rl_repo" not in sys.path:
    sys.path.insert(0, "/opt/trn_rl_repo")

import numpy as np
import ml_dtypes

import concourse.bass as bass
import concourse.tile as tile
from concourse import bacc, mybir
from concourse.bass_utils import run_bass_kernel_spmd
from concourse.masks import make_identity

F32 = mybir.dt.float32
F16 = mybir.dt.float16

NCORES = 8
B, R, C, O, I = 256, 1152, 10, 16, 8
RL = R // NCORES          # 144 routes per core
RI = RL * I               # 1152 (r,i) rows per core
NT = RI // 128            # 9 K-chunks of 128
CO = C * O                # 160
BH = B // 128             # 2 batch half-tiles
H = RI // 3               # 384: M-matmul free chunk
GROUPS = [(0, 1, 2, 3), (4, 5, 6, 7), (8, 9)]   # col-tiled capsule groups

AP = bass.AP


def _insert_bcast(base, pos, count):
    """Insert a step-0 (broadcast) free dim into an existing AP at index pos."""
    dims = list(base.ap)
    dims.insert(pos, [0, count])
    return AP(tensor=base.tensor, offset=base.offset, ap=dims)


def build_kernel(n_iters: int, collectives: bool = True):
    nc = bacc.Bacc("TRN2", target_bir_lowering=False, debug=False,
                   num_devices=NCORES)

    xt_in = nc.dram_tensor("xt", [128, NT, B], F32, kind="ExternalInput")
    xb_in = nc.dram_tensor("xb", [128, BH, RI], F16, kind="ExternalInput")
    ws_in = nc.dram_tensor("ws", [128, NT, CO], F32, kind="ExternalInput")
    wot_in = nc.dram_tensor("wot", [16, C, RI], F16, kind="ExternalInput")
    out = nc.dram_tensor("out", [B, CO], F32, kind="ExternalOutput")

    with tile.TileContext(nc) as tc:
        with (
            tc.tile_pool(name="stat", bufs=1) as stat,
            tc.tile_pool(name="work", bufs=2) as work,
            tc.tile_pool(name="sm", bufs=1) as smp,
            tc.tile_pool(name="ent", bufs=5) as entp,
            tc.tile_pool(name="ytp", bufs=3) as ytp,
            tc.tile_pool(name="mtp", bufs=4) as mtp,
            tc.tile_pool(name="dram", bufs=2, space="DRAM") as dram,
            tc.tile_pool(name="ps_m", bufs=3, space="PSUM") as ps_m,
            tc.tile_pool(name="ps_t", bufs=1, space="PSUM") as ps_t,
            tc.tile_pool(name="ps_s", bufs=2, space="PSUM") as ps_s,
        ):
            def _copy(eng, dst, src):
                if eng is nc.scalar:
                    eng.copy(dst, src)
                else:
                    eng.tensor_copy(dst, src)
            # ---- dummy warmup collective (absorbs entry barrier) ----
            dz = stat.tile([1, 4], F32)
            if collectives:
                d_in = dram.tile([1, 4], F32, tag="d_in")
                d_out = dram.tile([1, 4], F32, tag="d_out")
                nc.vector.memset(dz, 0.0)
                nc.sync.dma_start(out=d_in[:, :], in_=dz[:, :])
                nc.gpsimd.collective_compute(
                    "AllReduce", mybir.AluOpType.add,
                    replica_groups=[list(range(NCORES))],
                    ins=[d_in[:].opt()], outs=[d_out[:].opt()],
                )
                nc.sync.dma_start(out=dz[:, :], in_=d_out[:, :])
            else:
                nc.vector.memset(dz, 0.0)

            # ---- static SBUF tensors ----
            XT = stat.tile([128, NT, B], F32)        # x^T [(r,i)%128, t, b]
            XB = stat.tile([128, BH, RI], F16)       # x   [b%128, bh, (r,i)]
            WS = stat.tile([128, NT, CO], F32)       # W as lhsT for s-matmul
            WOTB = stat.tile([16, C, RI], F16)      # W^T bf16 rhs for M-mm
            XTB = stat.tile([128, NT, B], F16)
            WSB = stat.tile([128, NT, CO], F16)
            IDB = stat.tile([128, 128], F16)
            IDF = stat.tile([128, 128], F32)
            nc.sync.dma_start(out=XT, in_=xt_in[:])
            nc.scalar.dma_start(out=XB, in_=xb_in[:])
            nc.sync.dma_start(out=WS, in_=ws_in[:])
            nc.scalar.dma_start(out=WOTB, in_=wot_in[:])
            nc.vector.tensor_copy(XTB[:, :, :], XT[:, :, :])
            nc.vector.tensor_copy(WSB[:, :, :], WS[:, :, :])
            make_identity(nc, IDB[:, :])
            make_identity(nc, IDF[:, :])

            # logits b_ij, layout [p=b%128, (bh, c, r)]
            blog = stat.tile([128, BH, C, RL], F32)

            # s / v in [co, b] layout: two partition tiles (128 + 32 rows)
            sA = stat.tile([128, B], F32)            # co 0..127
            sB = stat.tile([32, B], F32)             # co 128..159
            sAb = stat.tile([128, B], F16)
            sBb = stat.tile([32, B], F16)
            vT = stat.tile([16, C, B], F16)         # v^T [o, c, b] bf16

            def s0_matmul():
                """s0 partials: psum [co,b] f32 (two tiles)."""
                p1t = ps_t.tile([128, B], F32, tag="ep1")
                p2t = ps_t.tile([32, B], F32, tag="ep2")
                p1 = p1t[:, :]
                p2 = p2t[:, :]
                for t in range(NT):
                    xcol = XT[:, t, :]
                    nc.tensor.matmul(p1, WS[:, t, 0:128], xcol,
                                     start=(t == 0), stop=(t == NT - 1))
                    nc.tensor.matmul(p2, WS[:, t, 128:160], xcol,
                                     start=(t == 0), stop=(t == NT - 1))
                nc.scalar.copy(sA[:, :], p1)
                nc.scalar.copy(sB[:, :], p2)

            def allreduce_s(first, st=None):
                """bounce -> AllReduce -> back to sA/sB [co, b] tiles.

                first: source is sA/sB (s0 path), bounce layout [CO, B].
                else: source is st [16(o), C, B], bounce layout [O, C, B];
                the return DMA scatters (c,o)-major back into sA/sB."""
                if first:
                    b_in = dram.tile([CO, B], F32, tag="arin")
                    b_out = dram.tile([CO, B], F32, tag="arout")
                    if collectives:
                        # liveness tie for the warmup collective (zeros)
                        nc.vector.tensor_add(sA[0:1, 0:4], sA[0:1, 0:4],
                                             dz[0:1, 0:4])
                    nc.sync.dma_start(out=b_in[0:128, :], in_=sA[:, :])
                    nc.scalar.dma_start(out=b_in[128:160, :], in_=sB[:, :])
                else:
                    b_in = dram.tile([O, C, B], F32, tag="arin2")
                    b_out = dram.tile([O, C, B], F32, tag="arout2")
                    base = b_in[:]
                    for j in range(4):
                        cnt = len([1 for grp in GROUPS if len(grp) > j])
                        dst = AP(tensor=base.tensor,
                                 offset=base.offset + j * B,
                                 ap=[[C * B, O], [4 * B, cnt], [1, B]])
                        qeng = nc.sync if (j % 2 == 0) else nc.scalar
                        qeng.dma_start(out=dst,
                                       in_=st[32 * j:32 * j + 16, 0:cnt, :])
                if collectives:
                    nc.gpsimd.collective_compute(
                        "AllReduce", mybir.AluOpType.add,
                        replica_groups=[list(range(NCORES))],
                        ins=[b_in[:].opt()], outs=[b_out[:].opt()],
                    )
                else:
                    nc.sync.dma_start(out=b_out[:], in_=b_in[:])
                if first:
                    nc.sync.dma_start(out=sA[:, :], in_=b_out[0:128, :])
                    nc.scalar.dma_start(out=sB[:, :], in_=b_out[128:160, :])
                else:
                    co_b = b_out[:].rearrange("o c b -> c o b")
                    nc.sync.dma_start(out=sA[:, :], in_=co_b[0:8, :, :])
                    nc.scalar.dma_start(out=sB[:, :], in_=co_b[8:10, :, :])

            def squash(scale, last):
                """v = s*|s|/(1+s^2) elementwise on [co,b] tiles (in-place).
                Produces bf16 copies + v^T realign unless last."""
                for s, sb in ((sA, sAb), (sB, sBb)):
                    sq = work.tile(list(s.shape), F32, tag=f"sq{s.shape[0]}")
                    ab = work.tile(list(s.shape), F32, tag=f"ab{s.shape[0]}")
                    sf = s[:, :]
                    if scale != 1.0:
                        nc.scalar.mul(sf, sf, scale)
                    nc.scalar.square(sq[:, :], sf)
                    nc.scalar.sqrt(ab[:, :], sq[:, :])
                    nc.vector.tensor_scalar_add(sq[:, :], sq[:, :], 1.0)
                    nc.vector.reciprocal_approx_fast(sq[:, :], sq[:, :])
                    nc.vector.tensor_mul(ab[:, :], ab[:, :], sq[:, :])
                    nc.vector.tensor_mul(sf, ab[:, :], sf)
                    if not last:
                        nc.vector.tensor_copy(sb[:, :], s[:, :])
                if not last:
                    for c in range(C):
                        src = sAb[c * 16:(c + 1) * 16, :] if c < 8 else \
                            sBb[(c - 8) * 16:(c - 7) * 16, :]
                        qeng = nc.sync if (c % 2 == 0) else nc.scalar
                        qeng.dma_start(out=vT[:, c, :], in_=src)

            def a_phase(first):
                """blog (+)= a;  a_c[b,r] = sum_i x*M, M = v_c @ WoT_c.

                M stays in PSUM (f32); the x*M product reads PSUM directly
                on DVE for most capsules; a few capsules go through a
                scalar-drained fp16 staging copy so GpSimd (no PSUM port)
                can carry part of the multiply load."""
                ar = smp.tile([128, BH, C, RL], F32, tag="ared")
                dst = blog if first else ar
                for c in range(C):
                    for bh in range(BH):
                        # M chunks drain psum->SBUF on Scalar only, so the
                        # PE's M-matmuls stream back-to-back (ps_m ring) and
                        # HAM stays warm; the x*M mul reads SBUF fp16.
                        prod = mtp.tile([128, RI], F32, tag="prod")
                        mt = mtp.tile([128, RI], F16, tag="mtmp")
                        lhs = vT[:, c, bh * 128:(bh + 1) * 128]
                        for h in range(3):
                            mp = ps_m.tile([128, H], F32, tag="mpsum")
                            nc.tensor.matmul(mp[:, :], lhs,
                                             WOTB[:, c, h * H:(h + 1) * H],
                                             start=True, stop=True)
                            sl = slice(h * H, (h + 1) * H)
                            nc.scalar.copy(mt[:, sl], mp[:, :])
                        peng = nc.gpsimd if c in (2, 6) else nc.vector
                        peng.tensor_mul(prod[:, :], mt[:, :], XB[:, bh, :])
                        tv = prod[:, :].rearrange("p (r i) -> p r i", i=I)
                        if c in (1, 5, 9):
                            # gpsimd tree-reduce over i (offloads DVE)
                            w1 = mtp.tile([128, RL, 4], F32, tag="tr")
                            nc.gpsimd.tensor_add(w1[:, :, :], tv[:, :, 0:4],
                                                 tv[:, :, 4:8])
                            nc.gpsimd.tensor_add(w1[:, :, 0:2], w1[:, :, 0:2],
                                                 w1[:, :, 2:4])
                            nc.gpsimd.tensor_add(dst[:, bh, c, :],
                                                 w1[:, :, 0], w1[:, :, 1])
                        else:
                            nc.vector.tensor_reduce(dst[:, bh, c, :], tv,
                                                    axis=mybir.AxisListType.X,
                                                    op=mybir.AluOpType.add)
                if not first:
                    nc.vector.tensor_add(blog[:, :, :, :], blog[:, :, :, :],
                                         ar[:, :, :, :])

            def softmax_en():
                """en = softmax_c(blog) in fp16, pipelined per b-half."""
                mx = smp.tile([128, BH, RL], F32, tag="mx")
                e = smp.tile([128, BH, C, RL], F32, tag="e")
                z = smp.tile([128, BH, RL], F32, tag="z")
                en = smp.tile([128, BH, C, RL], F16, tag="en")
                for bh in range(BH):
                    # shift by max over c (in place: softmax-invariant, and
                    # the shift persists harmlessly across iterations)
                    bv = blog[:, bh, :, :].rearrange("p c r -> p r c")
                    nc.vector.tensor_reduce(mx[:, bh, :], bv,
                                            axis=mybir.AxisListType.X,
                                            op=mybir.AluOpType.max)
                    mxb = _insert_bcast(mx[:, bh, :], 1, C)
                    nc.gpsimd.tensor_sub(blog[:, bh, :, :], blog[:, bh, :, :],
                                         mxb)
                    nc.scalar.activation(e[:, bh, :, :], blog[:, bh, :, :],
                                         mybir.ActivationFunctionType.Exp)
                    ev = e[:, bh, :, :].rearrange("p c r -> p r c")
                    nc.vector.tensor_reduce(z[:, bh, :], ev,
                                            axis=mybir.AxisListType.X,
                                            op=mybir.AluOpType.add)
                    nc.vector.reciprocal_approx_fast(z[:, bh, :], z[:, bh, :])
                    zb = _insert_bcast(z[:, bh, :], 1, C)
                    nc.vector.tensor_mul(en[:, bh, :, :], e[:, bh, :, :], zb)
                return en

            def s_phase(en, last):
                """en -> enT (PE) -> replicate (DMA) -> y -> s psum -> st4.

                s-matmuls are column-tiled: capsules go in groups of <=4,
                each to PE column strip 32j (out partitions 32j..32j+16),
                so up to 4 capsules' N=256 streams run concurrently on the
                otherwise 7/8-idle array. st4[32j+o, g, b] holds capsule
                c = 4g+j."""
                st4 = smp.tile([128, 3, B], F32, tag="st4")
                for g in range(3):
                    grp = GROUPS[g]
                    spg = ps_s.tile([128, B], F32, tag="spc")
                    for j, c in enumerate(grp):
                        ep1 = ps_t.tile([128, B], F32, tag="ep1")
                        ep2 = ps_t.tile([32, B], F32, tag="ep2")
                        for bh in range(BH):
                            cols = slice(bh * 128, (bh + 1) * 128)
                            nc.tensor.matmul(ep1[:, cols],
                                             en[:, bh, c, 0:128],
                                             IDB[:, :], start=True, stop=True)
                            nc.tensor.matmul(ep2[0:16, cols],
                                             en[:, bh, c, 128:RL],
                                             IDB[:, :], start=True, stop=True)
                        et1 = entp.tile([128, B], F16, tag="et1")
                        et2 = entp.tile([16, B], F16, tag="et2")
                        nc.scalar.copy(et1[:, :], ep1[:, :])
                        nc.scalar.copy(et2[:, :], ep2[0:16, :])
                        etr = ytp.tile([128, NT, B], F16, tag="etr")
                        for t in range(NT):
                            if t < 8:
                                base = et1[16 * t:16 * t + 16, :]
                            else:
                                base = et2[0:16, :]
                            src = _insert_bcast(base, 1, I)
                            qeng = nc.sync if (t % 2 == 0) else nc.scalar
                            qeng.dma_start(out=etr[:, t, :], in_=src)
                        sl = slice(32 * j, 32 * j + 16)
                        meng = nc.gpsimd if c in (3, 7) else nc.vector
                        if last:
                            ytc = ytp.tile([128, NT, B], F16, tag="ytcb")
                            ysrc, wsrc = XTB, WSB
                        else:
                            ytc = ytp.tile([128, NT, B], F32, tag="ytcf")
                            ysrc, wsrc = XT, WS
                        # 3-chunk mul: t-chunk k unblocks its s-matmuls
                        # before the later replicate DMAs finish
                        for k in range(3):
                            ts = slice(3 * k, 3 * k + 3)
                            meng.tensor_mul(ytc[:, ts, :], etr[:, ts, :],
                                            ysrc[:, ts, :])
                        for t in range(NT):
                            nc.tensor.matmul(
                                spg[sl, :],
                                wsrc[:, t, c * 16:(c + 1) * 16],
                                ytc[:, t, :], start=(t == 0),
                                stop=(t == NT - 1),
                                tile_position=(0, 32 * j))
                        nc.scalar.copy(st4[sl, g, :], spg[sl, :])
                return st4

            def emit_output():
                """v [co,b] -> out [b, co] via PE transpose."""
                ob = work.tile([128, BH, CO], F32, tag="ob")
                for bh in range(BH):
                    po = ps_m.tile([128, H], F32, tag="mpsum")
                    cols = slice(bh * 128, (bh + 1) * 128)
                    nc.tensor.matmul(po[:, 0:128], sA[:, cols], IDF[:, :],
                                     start=True, stop=True)
                    nc.tensor.matmul(po[:, 128:160], sB[:, cols],
                                     IDF[0:32, 0:32], start=True, stop=True)
                    nc.scalar.copy(ob[:, bh, :], po[:, 0:160])
                dst = out[:].rearrange("(bh p) co -> p bh co", p=128)
                nc.sync.dma_start(out=dst, in_=ob[:, :, :])

            # ---------------- routing ----------------
            s0_matmul()
            allreduce_s(first=True)
            squash(0.1, last=(n_iters == 1))
            for it in range(1, n_iters):
                last = (it == n_iters - 1)
                a_phase(first=(it == 1))
                en = softmax_en()
                st = s_phase(en, last)
                allreduce_s(first=False, st=st)
                squash(1.0, last=last)
            emit_output()

    nc.compile()
    return nc


def prep_inputs(x: np.ndarray, W: np.ndarray):
    """Host-side layout prep. Returns per-core input dicts."""
    W = W[0]  # [R, C, O, I]
    in_maps = []
    for k in range(NCORES):
        rs = slice(k * RL, (k + 1) * RL)
        xk = np.ascontiguousarray(x[:, rs, :])      # [B, RL, I]
        wk = np.ascontiguousarray(W[rs])            # [RL, C, O, I]
        xt = np.transpose(xk, (1, 2, 0)).reshape(NT, 128, B)
        xt = np.transpose(xt, (1, 0, 2))            # [128, NT, B]
        xb = xk.reshape(BH, 128, RI)
        xb = np.transpose(xb, (1, 0, 2))            # [128, BH, RI]
        # ws[p, t, c*16+o] = W[16t + p//8, c, o, p%8]
        wsk = np.transpose(wk.reshape(NT, 16, C, O, I), (0, 1, 4, 2, 3))
        wsk = wsk.reshape(NT, 128, CO)
        wsk = np.transpose(wsk, (1, 0, 2))          # [128, NT, CO]
        # wot[o, c, r*8+i] = W[r, c, o, i]
        wotk = np.transpose(wk, (2, 1, 0, 3)).reshape(O, C, RI)
        f32 = np.float32
        in_maps.append({
            "xt": np.ascontiguousarray(xt).astype(f32),
            "xb": np.ascontiguousarray(xb).astype(np.float16),
            "ws": np.ascontiguousarray(wsk).astype(f32),
            "wot": np.ascontiguousarray(wotk).astype(np.float16),
        })
    return in_maps


_CACHE = {}


def _get_nc(n_iters: int):
    if n_iters not in _CACHE:
        _CACHE[n_iters] = build_kernel(n_iters)
    return _CACHE[n_iters]


def kernel(x, W, num_iterations, _trace=False):
    n = int(num_iterations)
    assert n >= 1
    nc = _get_nc(n)
    in_maps = prep_inputs(np.asarray(x, dtype=np.float32),
                          np.asarray(W, dtype=np.float32))
    res = run_bass_kernel_spmd(nc, in_maps, list(range(NCORES)),
                               trace=_trace)
    v = res.results[0]["out"].reshape(B, C, O, 1).astype(np.float32)
    kernel.last_results = res
    return v


# revision 46
# speedup vs baseline: 1.2193x; 1.2193x over previous
"""DigitCaps dynamic-routing kernel for 8 TRN2 NeuronCores (v2).

Problem (hardcoded): x [256,1152,8] f32, W [1,1152,10,16,8] f32, 3 routing
iterations -> v [256,10,16,1] f32.

Strategy: shard the R=1152 routes 8-ways (144 per core), full batch B=256 on
every core. u_hat is never materialized; each iteration streams W through the
TensorEngine:
  s_c[o,b]   = sum_{(r,i)} Ws_c[(r,i),(c,o)] * (en_c[r,b] * x[(r,i),b])  (PE)
  (AllReduce s over the 8 R-shards in [CO,B] layout, squash -> v)
  M_c[b,(r,i)] = sum_o v_c[o,b] * WoT_c[o,(r,i)]                         (PE)
  a_c[b,r]   = sum_i x[b,(r,i)] * M_c[b,(r,i)]                           (DVE)

v2 changes vs baseline:
- AllReduce bounce kept in [CO,B] layout (contiguous descriptors; the old
  transposed write emitted ~41k 4-byte descriptors / 100us per phase).
- v lives in [co,b]; v^T for the M-matmul is 10 small realign DMAs; the
  whole v_transpose PE phase is gone. Output transposed once at the end.
- softmax without max-shift (logits are bounded ~+-30, exp is f32-safe);
  z-reciprocal applied via step-0 broadcast AP (no zrep materialization).
- en in bf16 (measured end-to-end impact ~5e-3); en-transpose via normal
  matmul against a bf16 identity (1cyc/row vs 4 for fp32 LOW_HIGH).
- M-path (a-phase) in bf16, s-path f32 except the last iteration (bf16);
  measured combined rel err ~6e-3 vs the 2e-2 gate.
- dummy 4-byte AllReduce issued first to absorb the cc entry barrier /
  ncfw warmup under the input load.
- engine spread: psum drains and big elementwise ops split across
  Scalar/Vector/GpSimd so no single engine serializes; GpSimd kept free
  near collective triggers.
"""

import sys

if "/opt/trn_rl_repo" not in sys.path:
    sys.path.insert(0, "/opt/trn_rl_repo")

import numpy as np
import ml_dtypes

import concourse.bass as bass
import concourse.tile as tile
from concourse import bacc, mybir
from concourse.bass_utils import run_bass_kernel_spmd
from concourse.masks import make_identity

F32 = mybir.dt.float32
F16 = mybir.dt.float16

NCORES = 8
B, R, C, O, I = 256, 1152, 10, 16, 8
RL = R // NCORES          # 144 routes per core
RI = RL * I               # 1152 (r,i) rows per core
NT = RI // 128            # 9 K-chunks of 128
CO = C * O                # 160
BH = B // 128             # 2 batch half-tiles
H = RI // 3               # 384: M-matmul free chunk
GROUPS = [(0, 1, 2, 3), (4, 5, 6, 7), (8, 9)]   # col-tiled capsule groups

AP = bass.AP


def _insert_bcast(base, pos, count):
    """Insert a step-0 (broadcast) free dim into an existing AP at index pos."""
    dims = list(base.ap)
    dims.insert(pos, [0, count])
    return AP(tensor=base.tensor, offset=base.offset, ap=dims)


def build_kernel(n_iters: int, collectives: bool = True):
    nc = bacc.Bacc("TRN2", target_bir_lowering=False, debug=False,
                   num_devices=NCORES)

    xt_in = nc.dram_tensor("xt", [128, NT, B], F32, kind="ExternalInput")
    xb_in = nc.dram_tensor("xb", [128, BH, RI], F16, kind="ExternalInput")
    ws_in = nc.dram_tensor("ws", [128, NT, CO], F32, kind="ExternalInput")
    wot_in = nc.dram_tensor("wot", [16, C, RI], F16, kind="ExternalInput")
    out = nc.dram_tensor("out", [B, CO], F32, kind="ExternalOutput")

    with tile.TileContext(nc) as tc:
        with (
            tc.tile_pool(name="stat", bufs=1) as stat,
            tc.tile_pool(name="work", bufs=2) as work,
            tc.tile_pool(name="sm", bufs=1) as smp,
            tc.tile_pool(name="ent", bufs=5) as entp,
            tc.tile_pool(name="ytp", bufs=3) as ytp,
            tc.tile_pool(name="mtp", bufs=4) as mtp,
            tc.tile_pool(name="dram", bufs=2, space="DRAM") as dram,
            tc.tile_pool(name="ps_m", bufs=3, space="PSUM") as ps_m,
            tc.tile_pool(name="ps_t", bufs=1, space="PSUM") as ps_t,
            tc.tile_pool(name="ps_s", bufs=2, space="PSUM") as ps_s,
        ):
            def _copy(eng, dst, src):
                if eng is nc.scalar:
                    eng.copy(dst, src)
                else:
                    eng.tensor_copy(dst, src)
            # ---- static SBUF tensors ----
            XT = stat.tile([128, NT, B], F32)        # x^T [(r,i)%128, t, b]
            XB = stat.tile([128, BH, RI], F16)       # x   [b%128, bh, (r,i)]
            WS = stat.tile([128, NT, CO], F32)       # W as lhsT for s-matmul
            WOTB = stat.tile([16, C, RI], F16)      # W^T bf16 rhs for M-mm
            XTB = stat.tile([128, NT, B], F16)
            WSB = stat.tile([128, NT, CO], F16)
            IDB = stat.tile([128, 128], F16)
            IDF = stat.tile([128, 128], F32)
            nc.sync.dma_start(out=XT, in_=xt_in[:])
            nc.scalar.dma_start(out=XB, in_=xb_in[:])
            nc.sync.dma_start(out=WS, in_=ws_in[:])
            nc.scalar.dma_start(out=WOTB, in_=wot_in[:])
            nc.vector.tensor_copy(XTB[:, :, :], XT[:, :, :])
            nc.vector.tensor_copy(WSB[:, :, :], WS[:, :, :])
            make_identity(nc, IDB[:, :])
            make_identity(nc, IDF[:, :])

            # logits b_ij, layout [p=b%128, (bh, c, r)]
            blog = stat.tile([128, BH, C, RL], F32)

            # s / v in [co, b] layout: two partition tiles (128 + 32 rows)
            sA = stat.tile([128, B], F32)            # co 0..127
            sB = stat.tile([32, B], F32)             # co 128..159
            sAb = stat.tile([128, B], F16)
            sBb = stat.tile([32, B], F16)
            vT = stat.tile([16, C, B], F16)         # v^T [o, c, b] bf16

            def s0_matmul():
                """s0 partials: psum [co,b] f32 (two tiles)."""
                p1t = ps_t.tile([128, B], F32, tag="ep1")
                p2t = ps_t.tile([32, B], F32, tag="ep2")
                p1 = p1t[:, :]
                p2 = p2t[:, :]
                for t in range(NT):
                    xcol = XT[:, t, :]
                    nc.tensor.matmul(p1, WS[:, t, 0:128], xcol,
                                     start=(t == 0), stop=(t == NT - 1))
                    nc.tensor.matmul(p2, WS[:, t, 128:160], xcol,
                                     start=(t == 0), stop=(t == NT - 1))
                nc.scalar.copy(sA[:, :], p1)
                nc.scalar.copy(sB[:, :], p2)

            def allreduce_s(first, st=None):
                """bounce -> AllReduce -> back to sA/sB [co, b] tiles.

                first: source is sA/sB (s0 path), bounce layout [CO, B].
                else: source is st [16(o), C, B], bounce layout [O, C, B];
                the return DMA scatters (c,o)-major back into sA/sB."""
                if first:
                    b_in = dram.tile([CO, B], F32, tag="arin")
                    b_out = dram.tile([CO, B], F32, tag="arout")
                    nc.sync.dma_start(out=b_in[0:128, :], in_=sA[:, :])
                    nc.scalar.dma_start(out=b_in[128:160, :], in_=sB[:, :])
                else:
                    b_in = dram.tile([O, C, B], F32, tag="arin2")
                    b_out = dram.tile([O, C, B], F32, tag="arout2")
                    base = b_in[:]
                    for j in range(4):
                        cnt = len([1 for grp in GROUPS if len(grp) > j])
                        dst = AP(tensor=base.tensor,
                                 offset=base.offset + j * B,
                                 ap=[[C * B, O], [4 * B, cnt], [1, B]])
                        qeng = nc.sync if (j % 2 == 0) else nc.scalar
                        qeng.dma_start(out=dst,
                                       in_=st[32 * j:32 * j + 16, 0:cnt, :])
                if collectives:
                    nc.gpsimd.collective_compute(
                        "AllReduce", mybir.AluOpType.add,
                        replica_groups=[list(range(NCORES))],
                        ins=[b_in[:].opt()], outs=[b_out[:].opt()],
                    )
                else:
                    nc.sync.dma_start(out=b_out[:], in_=b_in[:])
                if first:
                    nc.sync.dma_start(out=sA[:, :], in_=b_out[0:128, :])
                    nc.scalar.dma_start(out=sB[:, :], in_=b_out[128:160, :])
                else:
                    co_b = b_out[:].rearrange("o c b -> c o b")
                    nc.sync.dma_start(out=sA[:, :], in_=co_b[0:8, :, :])
                    nc.scalar.dma_start(out=sB[:, :], in_=co_b[8:10, :, :])

            def squash(scale, last):
                """v = s*|s|/(1+s^2) elementwise on [co,b] tiles (in-place).
                Produces bf16 copies + v^T realign unless last."""
                for s, sb in ((sA, sAb), (sB, sBb)):
                    sq = work.tile(list(s.shape), F32, tag=f"sq{s.shape[0]}")
                    ab = work.tile(list(s.shape), F32, tag=f"ab{s.shape[0]}")
                    sf = s[:, :]
                    if scale != 1.0:
                        nc.scalar.mul(sf, sf, scale)
                    nc.scalar.square(sq[:, :], sf)
                    nc.scalar.sqrt(ab[:, :], sq[:, :])
                    nc.vector.tensor_scalar_add(sq[:, :], sq[:, :], 1.0)
                    nc.vector.reciprocal_approx_fast(sq[:, :], sq[:, :])
                    nc.vector.tensor_mul(ab[:, :], ab[:, :], sq[:, :])
                    nc.vector.tensor_mul(sf, ab[:, :], sf)
                    if not last:
                        nc.vector.tensor_copy(sb[:, :], s[:, :])
                if not last:
                    for c in range(C):
                        src = sAb[c * 16:(c + 1) * 16, :] if c < 8 else \
                            sBb[(c - 8) * 16:(c - 7) * 16, :]
                        qeng = nc.sync if (c % 2 == 0) else nc.scalar
                        qeng.dma_start(out=vT[:, c, :], in_=src)

            def a_phase(first):
                """blog (+)= a;  a_c[b,r] = sum_i x*M, M = v_c @ WoT_c.

                M stays in PSUM (f32); the x*M product reads PSUM directly
                on DVE for most capsules; a few capsules go through a
                scalar-drained fp16 staging copy so GpSimd (no PSUM port)
                can carry part of the multiply load."""
                ar = smp.tile([128, BH, C, RL], F32, tag="ared")
                dst = blog if first else ar
                for c in range(C):
                    for bh in range(BH):
                        prod = mtp.tile([128, RI], F32, tag="prod")
                        lhs = vT[:, c, bh * 128:(bh + 1) * 128]
                        for h in range(3):
                            mp = ps_m.tile([128, H], F32, tag="mpsum")
                            nc.tensor.matmul(mp[:, :], lhs,
                                             WOTB[:, c, h * H:(h + 1) * H],
                                             start=True, stop=True)
                            sl = slice(h * H, (h + 1) * H)
                            nc.vector.tensor_mul(prod[:, sl],
                                                 mp[:, :], XB[:, bh, sl])
                        tv = prod[:, :].rearrange("p (r i) -> p r i", i=I)
                        if c in (1, 5, 9):
                            # gpsimd tree-reduce over i (offloads DVE)
                            w1 = mtp.tile([128, RL, 4], F32, tag="tr")
                            nc.gpsimd.tensor_add(w1[:, :, :], tv[:, :, 0:4],
                                                 tv[:, :, 4:8])
                            nc.gpsimd.tensor_add(w1[:, :, 0:2], w1[:, :, 0:2],
                                                 w1[:, :, 2:4])
                            nc.gpsimd.tensor_add(dst[:, bh, c, :],
                                                 w1[:, :, 0], w1[:, :, 1])
                        else:
                            nc.vector.tensor_reduce(dst[:, bh, c, :], tv,
                                                    axis=mybir.AxisListType.X,
                                                    op=mybir.AluOpType.add)
                if not first:
                    nc.vector.tensor_add(blog[:, :, :, :], blog[:, :, :, :],
                                         ar[:, :, :, :])

            def softmax_en():
                """en = softmax_c(blog) in fp16, pipelined per b-half."""
                mx = smp.tile([128, BH, RL], F32, tag="mx")
                e = smp.tile([128, BH, C, RL], F32, tag="e")
                z = smp.tile([128, BH, RL], F32, tag="z")
                en = smp.tile([128, BH, C, RL], F16, tag="en")
                for bh in range(BH):
                    # shift by max over c (in place: softmax-invariant, and
                    # the shift persists harmlessly across iterations)
                    bv = blog[:, bh, :, :].rearrange("p c r -> p r c")
                    nc.vector.tensor_reduce(mx[:, bh, :], bv,
                                            axis=mybir.AxisListType.X,
                                            op=mybir.AluOpType.max)
                    mxb = _insert_bcast(mx[:, bh, :], 1, C)
                    nc.gpsimd.tensor_sub(blog[:, bh, :, :], blog[:, bh, :, :],
                                         mxb)
                    nc.scalar.activation(e[:, bh, :, :], blog[:, bh, :, :],
                                         mybir.ActivationFunctionType.Exp)
                    ev = e[:, bh, :, :].rearrange("p c r -> p r c")
                    nc.vector.tensor_reduce(z[:, bh, :], ev,
                                            axis=mybir.AxisListType.X,
                                            op=mybir.AluOpType.add)
                    nc.vector.reciprocal_approx_fast(z[:, bh, :], z[:, bh, :])
                    zb = _insert_bcast(z[:, bh, :], 1, C)
                    nc.vector.tensor_mul(en[:, bh, :, :], e[:, bh, :, :], zb)
                return en

            def s_phase(en, last):
                """en -> enT (PE) -> replicate (DMA) -> y -> s psum -> st4.

                s-matmuls are column-tiled: capsules go in groups of <=4,
                each to PE column strip 32j (out partitions 32j..32j+16),
                so up to 4 capsules' N=256 streams run concurrently on the
                otherwise 7/8-idle array. st4[32j+o, g, b] holds capsule
                c = 4g+j."""
                st4 = smp.tile([128, 3, B], F32, tag="st4")
                for g in range(3):
                    grp = GROUPS[g]
                    spg = ps_s.tile([128, B], F32, tag="spc")
                    for j, c in enumerate(grp):
                        ep1 = ps_t.tile([128, B], F32, tag="ep1")
                        ep2 = ps_t.tile([32, B], F32, tag="ep2")
                        for bh in range(BH):
                            cols = slice(bh * 128, (bh + 1) * 128)
                            nc.tensor.matmul(ep1[:, cols],
                                             en[:, bh, c, 0:128],
                                             IDB[:, :], start=True, stop=True)
                            nc.tensor.matmul(ep2[0:16, cols],
                                             en[:, bh, c, 128:RL],
                                             IDB[:, :], start=True, stop=True)
                        et1 = entp.tile([128, B], F16, tag="et1")
                        et2 = entp.tile([16, B], F16, tag="et2")
                        nc.scalar.copy(et1[:, :], ep1[:, :])
                        nc.scalar.copy(et2[:, :], ep2[0:16, :])
                        etr = ytp.tile([128, NT, B], F16, tag="etr")
                        for t in range(NT):
                            if t < 8:
                                base = et1[16 * t:16 * t + 16, :]
                            else:
                                base = et2[0:16, :]
                            src = _insert_bcast(base, 1, I)
                            qeng = nc.sync if (t % 2 == 0) else nc.scalar
                            qeng.dma_start(out=etr[:, t, :], in_=src)
                        sl = slice(32 * j, 32 * j + 16)
                        meng = nc.gpsimd if c in (3, 7) else nc.vector
                        if last:
                            ytc = ytp.tile([128, NT, B], F16, tag="ytcb")
                            ysrc, wsrc = XTB, WSB
                        else:
                            ytc = ytp.tile([128, NT, B], F32, tag="ytcf")
                            ysrc, wsrc = XT, WS
                        # 3-chunk mul: t-chunk k unblocks its s-matmuls
                        # before the later replicate DMAs finish
                        for k in range(3):
                            ts = slice(3 * k, 3 * k + 3)
                            meng.tensor_mul(ytc[:, ts, :], etr[:, ts, :],
                                            ysrc[:, ts, :])
                        for t in range(NT):
                            nc.tensor.matmul(
                                spg[sl, :],
                                wsrc[:, t, c * 16:(c + 1) * 16],
                                ytc[:, t, :], start=(t == 0),
                                stop=(t == NT - 1),
                                tile_position=(0, 32 * j))
                        nc.scalar.copy(st4[sl, g, :], spg[sl, :])
                return st4

            def emit_output():
                """v [co,b] -> out [b, co] via PE transpose."""
                ob = work.tile([128, BH, CO], F32, tag="ob")
                for bh in range(BH):
                    po = ps_m.tile([128, H], F32, tag="mpsum")
                    cols = slice(bh * 128, (bh + 1) * 128)
                    nc.tensor.matmul(po[:, 0:128], sA[:, cols], IDF[:, :],
                                     start=True, stop=True)
                    nc.tensor.matmul(po[:, 128:160], sB[:, cols],
                                     IDF[0:32, 0:32], start=True, stop=True)
                    nc.scalar.copy(ob[:, bh, :], po[:, 0:160])
                dst = out[:].rearrange("(bh p) co -> p bh co", p=128)
                nc.sync.dma_start(out=dst, in_=ob[:, :, :])

            # ---------------- routing ----------------
            s0_matmul()
            allreduce_s(first=True)
            squash(0.1, last=(n_iters == 1))
            for it in range(1, n_iters):
                last = (it == n_iters - 1)
                a_phase(first=(it == 1))
                en = softmax_en()
                st = s_phase(en, last)
                allreduce_s(first=False, st=st)
                squash(1.0, last=last)
            emit_output()

    nc.compile()
    return nc


def prep_inputs(x: np.ndarray, W: np.ndarray):
    """Host-side layout prep. Returns per-core input dicts."""
    W = W[0]  # [R, C, O, I]
    in_maps = []
    for k in range(NCORES):
        rs = slice(k * RL, (k + 1) * RL)
        xk = np.ascontiguousarray(x[:, rs, :])      # [B, RL, I]
        wk = np.ascontiguousarray(W[rs])            # [RL, C, O, I]
        xt = np.transpose(xk, (1, 2, 0)).reshape(NT, 128, B)
        xt = np.transpose(xt, (1, 0, 2))            # [128, NT, B]
        xb = xk.reshape(BH, 128, RI)
        xb = np.transpose(xb, (1, 0, 2))            # [128, BH, RI]
        # ws[p, t, c*16+o] = W[16t + p//8, c, o, p%8]
        wsk = np.transpose(wk.reshape(NT, 16, C, O, I), (0, 1, 4, 2, 3))
        wsk = wsk.reshape(NT, 128, CO)
        wsk = np.transpose(wsk, (1, 0, 2))          # [128, NT, CO]
        # wot[o, c, r*8+i] = W[r, c, o, i]
        wotk = np.transpose(wk, (2, 1, 0, 3)).reshape(O, C, RI)
        f32 = np.float32
        in_maps.append({
            "xt": np.ascontiguousarray(xt).astype(f32),
            "xb": np.ascontiguousarray(xb).astype(np.float16),
            "ws": np.ascontiguousarray(wsk).astype(f32),
            "wot": np.ascontiguousarray(wotk).astype(np.float16),
        })
    return in_maps


_CACHE = {}


def _get_nc(n_iters: int):
    if n_iters not in _CACHE:
        _CACHE[n_iters] = build_kernel(n_iters)
    return _CACHE[n_iters]


def kernel(x, W, num_iterations, _trace=False):
    n = int(num_iterations)
    assert n >= 1
    nc = _get_nc(n)
    in_maps = prep_inputs(np.asarray(x, dtype=np.float32),
                          np.asarray(W, dtype=np.float32))
    res = run_bass_kernel_spmd(nc, in_maps, list(range(NCORES)),
                               trace=_trace)
    v = res.results[0]["out"].reshape(B, C, O, 1).astype(np.float32)
    kernel.last_results = res
    return v


# revision 48
# speedup vs baseline: 1.3894x; 1.1395x over previous
"""DigitCaps dynamic-routing kernel for 8 TRN2 NeuronCores (v2).

Problem (hardcoded): x [256,1152,8] f32, W [1,1152,10,16,8] f32, 3 routing
iterations -> v [256,10,16,1] f32.

Strategy: shard the R=1152 routes 8-ways (144 per core), full batch B=256 on
every core. u_hat is never materialized; each iteration streams W through the
TensorEngine:
  s_c[o,b]   = sum_{(r,i)} Ws_c[(r,i),(c,o)] * (en_c[r,b] * x[(r,i),b])  (PE)
  (AllReduce s over the 8 R-shards in [CO,B] layout, squash -> v)
  M_c[b,(r,i)] = sum_o v_c[o,b] * WoT_c[o,(r,i)]                         (PE)
  a_c[b,r]   = sum_i x[b,(r,i)] * M_c[b,(r,i)]                           (DVE)

v2 changes vs baseline:
- AllReduce bounce kept in [CO,B] layout (contiguous descriptors; the old
  transposed write emitted ~41k 4-byte descriptors / 100us per phase).
- v lives in [co,b]; v^T for the M-matmul is 10 small realign DMAs; the
  whole v_transpose PE phase is gone. Output transposed once at the end.
- softmax without max-shift (logits are bounded ~+-30, exp is f32-safe);
  z-reciprocal applied via step-0 broadcast AP (no zrep materialization).
- en in bf16 (measured end-to-end impact ~5e-3); en-transpose via normal
  matmul against a bf16 identity (1cyc/row vs 4 for fp32 LOW_HIGH).
- M-path (a-phase) in bf16, s-path f32 except the last iteration (bf16);
  measured combined rel err ~6e-3 vs the 2e-2 gate.
- dummy 4-byte AllReduce issued first to absorb the cc entry barrier /
  ncfw warmup under the input load.
- engine spread: psum drains and big elementwise ops split across
  Scalar/Vector/GpSimd so no single engine serializes; GpSimd kept free
  near collective triggers.
"""

import sys

if "/opt/trn_rl_repo" not in sys.path:
    sys.path.insert(0, "/opt/trn_rl_repo")

import numpy as np
import ml_dtypes

import concourse.bass as bass
import concourse.tile as tile
from concourse import bacc, mybir
from concourse.bass_utils import run_bass_kernel_spmd
from concourse.masks import make_identity

F32 = mybir.dt.float32
F16 = mybir.dt.float16

NCORES = 8
B, R, C, O, I = 256, 1152, 10, 16, 8
RL = R // NCORES          # 144 routes per core
RI = RL * I               # 1152 (r,i) rows per core
NT = RI // 128            # 9 K-chunks of 128
CO = C * O                # 160
BH = B // 128             # 2 batch half-tiles
H = RI // 3               # 384: M-matmul free chunk
GROUPS = [(0, 1, 2, 3), (4, 5, 6, 7), (8, 9)]   # col-tiled capsule groups

AP = bass.AP


def _insert_bcast(base, pos, count):
    """Insert a step-0 (broadcast) free dim into an existing AP at index pos."""
    dims = list(base.ap)
    dims.insert(pos, [0, count])
    return AP(tensor=base.tensor, offset=base.offset, ap=dims)


def build_kernel(n_iters: int, collectives: bool = True):
    nc = bacc.Bacc("TRN2", target_bir_lowering=False, debug=False,
                   num_devices=NCORES)

    xt_in = nc.dram_tensor("xt", [128, NT, B], F32, kind="ExternalInput")
    xb_in = nc.dram_tensor("xb", [128, BH, RI], F16, kind="ExternalInput")
    ws_in = nc.dram_tensor("ws", [128, NT, CO], F32, kind="ExternalInput")
    wot_in = nc.dram_tensor("wot", [16, C, RI], F16, kind="ExternalInput")
    out = nc.dram_tensor("out", [B, CO], F32, kind="ExternalOutput")

    with tile.TileContext(nc) as tc:
        with (
            tc.tile_pool(name="stat", bufs=1) as stat,
            tc.tile_pool(name="work", bufs=2) as work,
            tc.tile_pool(name="sm", bufs=1) as smp,
            tc.tile_pool(name="ent", bufs=5) as entp,
            tc.tile_pool(name="ytp", bufs=3) as ytp,
            tc.tile_pool(name="mtp", bufs=4) as mtp,
            tc.tile_pool(name="dram", bufs=2, space="DRAM") as dram,
            tc.tile_pool(name="ps_m", bufs=3, space="PSUM") as ps_m,
            tc.tile_pool(name="ps_t", bufs=1, space="PSUM") as ps_t,
            tc.tile_pool(name="ps_s", bufs=2, space="PSUM") as ps_s,
        ):
            def _copy(eng, dst, src):
                if eng is nc.scalar:
                    eng.copy(dst, src)
                else:
                    eng.tensor_copy(dst, src)

            # ---- dummy warmup collective: absorbs the cc entry barrier +
            # ncfw init under the input load, so AR#1 runs warm ----
            dz = stat.tile([1, 4], F32)
            if collectives:
                d_in = dram.tile([1, 4], F32, tag="d_in")
                d_out = dram.tile([1, 4], F32, tag="d_out")
                nc.vector.memset(dz, 0.0)
                nc.sync.dma_start(out=d_in[:, :], in_=dz[:, :])
                nc.gpsimd.collective_compute(
                    "AllReduce", mybir.AluOpType.add,
                    replica_groups=[list(range(NCORES))],
                    ins=[d_in[:].opt()], outs=[d_out[:].opt()],
                )
                nc.sync.dma_start(out=dz[:, :], in_=d_out[:, :])
            else:
                nc.vector.memset(dz, 0.0)

            # ---- static SBUF tensors ----
            XT = stat.tile([128, NT, B], F32)        # x^T [(r,i)%128, t, b]
            XB = stat.tile([128, BH, RI], F16)       # x   [b%128, bh, (r,i)]
            WS = stat.tile([128, NT, CO], F32)       # W as lhsT for s-matmul
            WOTB = stat.tile([16, C, RI], F16)      # W^T bf16 rhs for M-mm
            XTB = stat.tile([128, NT, B], F16)
            WSB = stat.tile([128, NT, CO], F16)
            IDB = stat.tile([128, 128], F16)
            IDF = stat.tile([128, 128], F32)
            nc.sync.dma_start(out=XT, in_=xt_in[:])
            nc.scalar.dma_start(out=XB, in_=xb_in[:])
            nc.sync.dma_start(out=WS, in_=ws_in[:])
            nc.scalar.dma_start(out=WOTB, in_=wot_in[:])
            nc.vector.tensor_copy(XTB[:, :, :], XT[:, :, :])
            nc.vector.tensor_copy(WSB[:, :, :], WS[:, :, :])
            make_identity(nc, IDB[:, :])
            make_identity(nc, IDF[:, :])

            # logits b_ij, layout [p=b%128, (bh, c, r)]
            blog = stat.tile([128, BH, C, RL], F32)

            # s / v in [co, b] layout: two partition tiles (128 + 32 rows)
            sA = stat.tile([128, B], F32)            # co 0..127
            sB = stat.tile([32, B], F32)             # co 128..159
            sAb = stat.tile([128, B], F16)
            sBb = stat.tile([32, B], F16)
            vT = stat.tile([16, C, B], F16)         # v^T [o, c, b] bf16

            def s0_matmul():
                """s0 partials: psum [co,b] f32 (two tiles)."""
                p1t = ps_t.tile([128, B], F32, tag="ep1")
                p2t = ps_t.tile([32, B], F32, tag="ep2")
                p1 = p1t[:, :]
                p2 = p2t[:, :]
                for t in range(NT):
                    xcol = XT[:, t, :]
                    nc.tensor.matmul(p1, WS[:, t, 0:128], xcol,
                                     start=(t == 0), stop=(t == NT - 1))
                    nc.tensor.matmul(p2, WS[:, t, 128:160], xcol,
                                     start=(t == 0), stop=(t == NT - 1))
                nc.scalar.copy(sA[:, :], p1)
                nc.scalar.copy(sB[:, :], p2)

            def allreduce_s(first, st=None):
                """bounce -> AllReduce -> back to sA/sB [co, b] tiles.

                first: source is sA/sB (s0 path), bounce layout [CO, B].
                else: source is st [16(o), C, B], bounce layout [O, C, B];
                the return DMA scatters (c,o)-major back into sA/sB."""
                if first:
                    b_in = dram.tile([CO, B], F32, tag="arin")
                    b_out = dram.tile([CO, B], F32, tag="arout")
                    if collectives:
                        # liveness tie for the warmup collective (zeros)
                        nc.vector.tensor_add(sA[0:1, 0:4], sA[0:1, 0:4],
                                             dz[0:1, 0:4])
                    nc.sync.dma_start(out=b_in[0:128, :], in_=sA[:, :])
                    nc.scalar.dma_start(out=b_in[128:160, :], in_=sB[:, :])
                else:
                    b_in = dram.tile([O, C, B], F32, tag="arin2")
                    b_out = dram.tile([O, C, B], F32, tag="arout2")
                    base = b_in[:]
                    for j in range(4):
                        cnt = len([1 for grp in GROUPS if len(grp) > j])
                        dst = AP(tensor=base.tensor,
                                 offset=base.offset + j * B,
                                 ap=[[C * B, O], [4 * B, cnt], [1, B]])
                        qeng = nc.sync if (j % 2 == 0) else nc.scalar
                        qeng.dma_start(out=dst,
                                       in_=st[32 * j:32 * j + 16, 0:cnt, :])
                if collectives:
                    nc.gpsimd.collective_compute(
                        "AllReduce", mybir.AluOpType.add,
                        replica_groups=[list(range(NCORES))],
                        ins=[b_in[:].opt()], outs=[b_out[:].opt()],
                    )
                else:
                    nc.sync.dma_start(out=b_out[:], in_=b_in[:])
                if first:
                    nc.sync.dma_start(out=sA[:, :], in_=b_out[0:128, :])
                    nc.scalar.dma_start(out=sB[:, :], in_=b_out[128:160, :])
                else:
                    co_b = b_out[:].rearrange("o c b -> c o b")
                    nc.sync.dma_start(out=sA[:, :], in_=co_b[0:8, :, :])
                    nc.scalar.dma_start(out=sB[:, :], in_=co_b[8:10, :, :])

            def squash(scale, last):
                """v = s*|s|/(1+s^2) elementwise on [co,b] tiles (in-place).
                Produces bf16 copies + v^T realign unless last."""
                for s, sb in ((sA, sAb), (sB, sBb)):
                    sq = work.tile(list(s.shape), F32, tag=f"sq{s.shape[0]}")
                    ab = work.tile(list(s.shape), F32, tag=f"ab{s.shape[0]}")
                    sf = s[:, :]
                    if scale != 1.0:
                        nc.scalar.mul(sf, sf, scale)
                    nc.scalar.square(sq[:, :], sf)
                    nc.scalar.sqrt(ab[:, :], sq[:, :])
                    nc.vector.tensor_scalar_add(sq[:, :], sq[:, :], 1.0)
                    nc.vector.reciprocal_approx_fast(sq[:, :], sq[:, :])
                    nc.vector.tensor_mul(ab[:, :], ab[:, :], sq[:, :])
                    nc.vector.tensor_mul(sf, ab[:, :], sf)
                    if not last:
                        nc.vector.tensor_copy(sb[:, :], s[:, :])
                if not last:
                    for c in range(C):
                        src = sAb[c * 16:(c + 1) * 16, :] if c < 8 else \
                            sBb[(c - 8) * 16:(c - 7) * 16, :]
                        qeng = nc.sync if (c % 2 == 0) else nc.scalar
                        qeng.dma_start(out=vT[:, c, :], in_=src)

            def a_phase(first):
                """blog (+)= a;  a_c[b,r] = sum_i x*M, M = v_c @ WoT_c.

                M stays in PSUM (f32); the x*M product reads PSUM directly
                on DVE for most capsules; a few capsules go through a
                scalar-drained fp16 staging copy so GpSimd (no PSUM port)
                can carry part of the multiply load."""
                ar = smp.tile([128, BH, C, RL], F32, tag="ared")
                dst = blog if first else ar
                for c in range(C):
                    for bh in range(BH):
                        prod = mtp.tile([128, RI], F32, tag="prod")
                        lhs = vT[:, c, bh * 128:(bh + 1) * 128]
                        for h in range(3):
                            mp = ps_m.tile([128, H], F32, tag="mpsum")
                            nc.tensor.matmul(mp[:, :], lhs,
                                             WOTB[:, c, h * H:(h + 1) * H],
                                             start=True, stop=True)
                            sl = slice(h * H, (h + 1) * H)
                            nc.vector.tensor_mul(prod[:, sl],
                                                 mp[:, :], XB[:, bh, sl])
                        tv = prod[:, :].rearrange("p (r i) -> p r i", i=I)
                        if c in (1, 5, 9):
                            # gpsimd tree-reduce over i (offloads DVE)
                            w1 = mtp.tile([128, RL, 4], F32, tag="tr")
                            nc.gpsimd.tensor_add(w1[:, :, :], tv[:, :, 0:4],
                                                 tv[:, :, 4:8])
                            nc.gpsimd.tensor_add(w1[:, :, 0:2], w1[:, :, 0:2],
                                                 w1[:, :, 2:4])
                            nc.gpsimd.tensor_add(dst[:, bh, c, :],
                                                 w1[:, :, 0], w1[:, :, 1])
                        else:
                            nc.vector.tensor_reduce(dst[:, bh, c, :], tv,
                                                    axis=mybir.AxisListType.X,
                                                    op=mybir.AluOpType.add)
                if not first:
                    nc.vector.tensor_add(blog[:, :, :, :], blog[:, :, :, :],
                                         ar[:, :, :, :])

            def softmax_en():
                """en = softmax_c(blog) in fp16, pipelined per b-half."""
                mx = smp.tile([128, BH, RL], F32, tag="mx")
                e = smp.tile([128, BH, C, RL], F32, tag="e")
                z = smp.tile([128, BH, RL], F32, tag="z")
                en = smp.tile([128, BH, C, RL], F16, tag="en")
                for bh in range(BH):
                    # shift by max over c (in place: softmax-invariant, and
                    # the shift persists harmlessly across iterations)
                    bv = blog[:, bh, :, :].rearrange("p c r -> p r c")
                    nc.vector.tensor_reduce(mx[:, bh, :], bv,
                                            axis=mybir.AxisListType.X,
                                            op=mybir.AluOpType.max)
                    mxb = _insert_bcast(mx[:, bh, :], 1, C)
                    nc.gpsimd.tensor_sub(blog[:, bh, :, :], blog[:, bh, :, :],
                                         mxb)
                    nc.scalar.activation(e[:, bh, :, :], blog[:, bh, :, :],
                                         mybir.ActivationFunctionType.Exp)
                    ev = e[:, bh, :, :].rearrange("p c r -> p r c")
                    nc.vector.tensor_reduce(z[:, bh, :], ev,
                                            axis=mybir.AxisListType.X,
                                            op=mybir.AluOpType.add)
                    nc.vector.reciprocal_approx_fast(z[:, bh, :], z[:, bh, :])
                    zb = _insert_bcast(z[:, bh, :], 1, C)
                    nc.vector.tensor_mul(en[:, bh, :, :], e[:, bh, :, :], zb)
                return en

            def s_phase(en, last):
                """en -> enT (PE) -> replicate (DMA) -> y -> s psum -> st4.

                s-matmuls are column-tiled: capsules go in groups of <=4,
                each to PE column strip 32j (out partitions 32j..32j+16),
                so up to 4 capsules' N=256 streams run concurrently on the
                otherwise 7/8-idle array. st4[32j+o, g, b] holds capsule
                c = 4g+j."""
                st4 = smp.tile([128, 3, B], F32, tag="st4")
                for g in range(3):
                    grp = GROUPS[g]
                    spg = ps_s.tile([128, B], F32, tag="spc")
                    for j, c in enumerate(grp):
                        ep1 = ps_t.tile([128, B], F32, tag="ep1")
                        ep2 = ps_t.tile([32, B], F32, tag="ep2")
                        for bh in range(BH):
                            cols = slice(bh * 128, (bh + 1) * 128)
                            nc.tensor.matmul(ep1[:, cols],
                                             en[:, bh, c, 0:128],
                                             IDB[:, :], start=True, stop=True)
                            nc.tensor.matmul(ep2[0:16, cols],
                                             en[:, bh, c, 128:RL],
                                             IDB[:, :], start=True, stop=True)
                        et1 = entp.tile([128, B], F16, tag="et1")
                        et2 = entp.tile([16, B], F16, tag="et2")
                        nc.scalar.copy(et1[:, :], ep1[:, :])
                        nc.scalar.copy(et2[:, :], ep2[0:16, :])
                        etr = ytp.tile([128, NT, B], F16, tag="etr")
                        for t in range(NT):
                            if t < 8:
                                base = et1[16 * t:16 * t + 16, :]
                            else:
                                base = et2[0:16, :]
                            src = _insert_bcast(base, 1, I)
                            qeng = nc.sync if (t % 2 == 0) else nc.scalar
                            qeng.dma_start(out=etr[:, t, :], in_=src)
                        sl = slice(32 * j, 32 * j + 16)
                        meng = nc.gpsimd if c in (3, 7) else nc.vector
                        if last:
                            ytc = ytp.tile([128, NT, B], F16, tag="ytcb")
                            ysrc, wsrc = XTB, WSB
                        else:
                            ytc = ytp.tile([128, NT, B], F32, tag="ytcf")
                            ysrc, wsrc = XT, WS
                        # 3-chunk mul: t-chunk k unblocks its s-matmuls
                        # before the later replicate DMAs finish
                        for k in range(3):
                            ts = slice(3 * k, 3 * k + 3)
                            meng.tensor_mul(ytc[:, ts, :], etr[:, ts, :],
                                            ysrc[:, ts, :])
                        for t in range(NT):
                            nc.tensor.matmul(
                                spg[sl, :],
                                wsrc[:, t, c * 16:(c + 1) * 16],
                                ytc[:, t, :], start=(t == 0),
                                stop=(t == NT - 1),
                                tile_position=(0, 32 * j))
                        nc.scalar.copy(st4[sl, g, :], spg[sl, :])
                return st4

            def emit_output():
                """v [co,b] -> out [b, co] via PE transpose."""
                ob = work.tile([128, BH, CO], F32, tag="ob")
                for bh in range(BH):
                    po = ps_m.tile([128, H], F32, tag="mpsum")
                    cols = slice(bh * 128, (bh + 1) * 128)
                    nc.tensor.matmul(po[:, 0:128], sA[:, cols], IDF[:, :],
                                     start=True, stop=True)
                    nc.tensor.matmul(po[:, 128:160], sB[:, cols],
                                     IDF[0:32, 0:32], start=True, stop=True)
                    nc.scalar.copy(ob[:, bh, :], po[:, 0:160])
                dst = out[:].rearrange("(bh p) co -> p bh co", p=128)
                nc.sync.dma_start(out=dst, in_=ob[:, :, :])

            # ---------------- routing ----------------
            s0_matmul()
            allreduce_s(first=True)
            squash(0.1, last=(n_iters == 1))
            for it in range(1, n_iters):
                last = (it == n_iters - 1)
                a_phase(first=(it == 1))
                en = softmax_en()
                st = s_phase(en, last)
                allreduce_s(first=False, st=st)
                squash(1.0, last=last)
            emit_output()

    nc.compile()
    return nc


def prep_inputs(x: np.ndarray, W: np.ndarray):
    """Host-side layout prep. Returns per-core input dicts."""
    W = W[0]  # [R, C, O, I]
    in_maps = []
    for k in range(NCORES):
        rs = slice(k * RL, (k + 1) * RL)
        xk = np.ascontiguousarray(x[:, rs, :])      # [B, RL, I]
        wk = np.ascontiguousarray(W[rs])            # [RL, C, O, I]
        xt = np.transpose(xk, (1, 2, 0)).reshape(NT, 128, B)
        xt = np.transpose(xt, (1, 0, 2))            # [128, NT, B]
        xb = xk.reshape(BH, 128, RI)
        xb = np.transpose(xb, (1, 0, 2))            # [128, BH, RI]
        # ws[p, t, c*16+o] = W[16t + p//8, c, o, p%8]
        wsk = np.transpose(wk.reshape(NT, 16, C, O, I), (0, 1, 4, 2, 3))
        wsk = wsk.reshape(NT, 128, CO)
        wsk = np.transpose(wsk, (1, 0, 2))          # [128, NT, CO]
        # wot[o, c, r*8+i] = W[r, c, o, i]
        wotk = np.transpose(wk, (2, 1, 0, 3)).reshape(O, C, RI)
        f32 = np.float32
        in_maps.append({
            "xt": np.ascontiguousarray(xt).astype(f32),
            "xb": np.ascontiguousarray(xb).astype(np.float16),
            "ws": np.ascontiguousarray(wsk).astype(f32),
            "wot": np.ascontiguousarray(wotk).astype(np.float16),
        })
    return in_maps


_CACHE = {}


def _get_nc(n_iters: int):
    if n_iters not in _CACHE:
        _CACHE[n_iters] = build_kernel(n_iters)
    return _CACHE[n_iters]


def kernel(x, W, num_iterations, _trace=False):
    n = int(num_iterations)
    assert n >= 1
    nc = _get_nc(n)
    in_maps = prep_inputs(np.asarray(x, dtype=np.float32),
                          np.asarray(W, dtype=np.float32))
    res = run_bass_kernel_spmd(nc, in_maps, list(range(NCORES)),
                               trace=_trace)
    v = res.results[0]["out"].reshape(B, C, O, 1).astype(np.float32)
    kernel.last_results = res
    return v
